# revision 42
# baseline (speedup 1.0000x reference)
# Trainium2 Bass kernel for nn_EquiRNBlock1 (gnn_message_passing).
#
# Reference computation (B=4, N=512, M=512, D=4, H=32, L=128):
#   pairs[b,n,m,d,:] = (Y[b,m,d], X[b,n,d])
#   h1 = relu(W1 @ pairs + b1)            # 2 -> 32, elementwise over (b,n,m,d)
#   h2 = relu(W2 @ h1 + b2)               # 32 -> 32
#   z  = w3 . h2 + b3                     # 32 -> 1
#   zs[b,n,m] = sum_d z                   # sum over D
#   e1 = relu(encw1 * zs + encb1)         # 1 -> 32
#   e2 = relu(encw2 @ e1 + encb2)         # 32 -> 32
#   e3 = encw3 @ e2 + encb3               # 32 -> 128
#   out[b,n,l] = max_m e3[b,n,m,l]
#
# Device mapping (8 cores, SPMD):
#   Flatten (b,n) -> 2048 rows; core c owns rows [256c, 256c+256) (each core
#   touches exactly one batch b = c//2).  Per (b,n) row the whole M=512 grid
#   is processed on-chip:
#     - Layer 1 splits algebraically: h1 = relu(U[b] + V[b,n]) with
#       U[(d,k),m] = W1[k,0]*Y[b,m,d] + b1[k] shared across rows and
#       V[(d,k)] = W1[k,1]*X[b,n,d] a per-partition scalar -> one GPSIMD
#       tensor_scalar (add, then max 0) per row.
#     - eq layer 2 = one 128x128 block-diag (over d) matmul per row.
#     - eq layer 3 + sum_d + enc layer 1 fold into one rank-1-combined
#       matmul; 4 rows pack into the 128 PSUM partitions via accumulation
#       of block-sparse weights (PE cannot write PSUM at a partition
#       offset, but it can accumulate full-height).
#     - enc layer 2 = one block-diag(4 rows) matmul per group.
#     - enc layer 3 reads the packed rows back out (rhs partition offsets
#       are legal) producing [L=128, m=512] per row; max over m = DVE
#       free-dim reduce, two rows per [128,1024] PSUM pair.
#   All matmul operands are float32r (TF32-like, 1 col/cycle at N>=256;
#   true fp32 streams at 1/4 rate).  Weights/activations round to f32r on
#   write, PSUM accumulates fp32.  Measured |rel err| ~6e-4 on hardware.
#   Outputs accumulate as [L, row] columns, PE-transposed at the end.

import numpy as np

B, N, M, D = 4, 512, 512, 4
H, L = 32, 128
NCORES = 8
ROWS = (B * N) // NCORES  # 256 rows per core
RG = 4                    # rows per pipeline group

_PROG = None
BF16_H1 = False
E2_ENGINE = "act"


def _build_program(loop_iters=None, h1_engine="dve", e2_engine="act", bf16_h1=False, ablate=None, streams=1, deep=False, h2pair=False):
    import contextlib
    import concourse.bacc as bacc
    import concourse.tile as tile
    import concourse.mybir as mybir

    f32 = mybir.dt.float32
    f32r = mybir.dt.float32r
    AF = mybir.ActivationFunctionType
    ALU = mybir.AluOpType
    AX = mybir.AxisListType

    import os
    nc = bacc.Bacc("TRN2", target_bir_lowering=False, debug=False, use_seq_codegen=os.environ.get("SEQCG", "") == "1")

    bf16 = mybir.dt.bfloat16
    uvdt = bf16 if bf16_h1 else f32
    w2dt = bf16 if bf16_h1 else f32r
    U = nc.dram_tensor("U", [128, M], uvdt, kind="ExternalInput").ap()
    V = nc.dram_tensor("V", [128, ROWS], f32, kind="ExternalInput").ap()
    W2BLK = nc.dram_tensor("W2BLK", [128, 128], w2dt, kind="ExternalInput").ap()
    WCOMBO4 = nc.dram_tensor("WCOMBO4", [128, 512], f32r, kind="ExternalInput").ap()
    W2BLK4 = nc.dram_tensor("W2BLK4", [128, 128], f32r, kind="ExternalInput").ap()
    ENC3T4 = nc.dram_tensor("ENC3T4", [128, 128], f32r, kind="ExternalInput").ap()
    BCOL = nc.dram_tensor("BCOL", [128, 4], f32, kind="ExternalInput").ap()
    IDN = nc.dram_tensor("IDN", [128, 128], f32, kind="ExternalInput").ap()
    OUT = nc.dram_tensor("OUT", [ROWS, 128], f32, kind="ExternalOutput").ap()

    with tile.TileContext(nc) as tc:
        with (
            tc.tile_pool(name="consts", bufs=1) as consts,
            tc.tile_pool(name="h1p", bufs=6) as h1pool,
            tc.tile_pool(name="h2rp", bufs=8) as h2rpool,
            tc.tile_pool(name="e1rp", bufs=3) as e1rpool,
            tc.tile_pool(name="e2rp", bufs=3) as e2rpool,
            tc.tile_pool(name="outp", bufs=1) as outpool,
            tc.tile_pool(name="psA", bufs=(1 if h2pair else (2 if streams == 1 else 1)),
                         space="PSUM") as psA,
            tc.tile_pool(name="psB", bufs=(2 if deep else 1), space="PSUM") as psB,
            tc.tile_pool(name="psC", bufs=(2 if deep else 1), space="PSUM") as psC,
            tc.tile_pool(name="psD", bufs=(2 if streams == 1 else 1),
                         space="PSUM") as psD,
            tc.tile_pool(name="psA2", bufs=1, space="PSUM") as psA2,
            tc.tile_pool(name="psB2", bufs=1, space="PSUM") as psB2,
            tc.tile_pool(name="psC2", bufs=1, space="PSUM") as psC2,
            tc.tile_pool(name="psD2", bufs=1, space="PSUM") as psD2,
        ):
            Usb = consts.tile_from(U, name="Usb")
            Vsb = consts.tile_from(V, name="Vsb")
            W2sb = consts.tile_from(W2BLK, name="W2sb")
            WC4sb = consts.tile_from(WCOMBO4, name="WC4sb")
            W24sb = consts.tile_from(W2BLK4, name="W24sb")
            E3sb = consts.tile_from(ENC3T4, name="E3sb")
            Bsb = consts.tile_from(BCOL, name="Bsb")
            IDsb = consts.tile_from(IDN, name="IDsb")
            B2sb = Bsb[:, 0:1]     # eq_b2 tiled over d
            B1sb = Bsb[:, 1:2]     # enc_w1*D*eq_b3 + enc_b1, tiled over rows
            B22sb = Bsb[:, 2:3]    # enc_b2 tiled over rows
            B3sb = Bsb[:, 3:4]     # enc_b3

            outacc = outpool.tile([128, ROWS], f32, name="outacc")

            loop_cm = (
                tc.For_i(0, loop_iters, 1,
                         hint_engines=(mybir.EngineType.PE,
                                       mybir.EngineType.Activation,
                                       mybir.EngineType.DVE,
                                       mybir.EngineType.Pool))
                if loop_iters is not None else contextlib.nullcontext()
            )
            with loop_cm:
                if streams == 2:
                    group_order = [2 * gp + s for gp in range(ROWS // RG // 2)
                                   for s in range(2)]
                else:
                    group_order = list(range(ROWS // RG))
                for g in group_order:
                    if streams == 2 and g % 2 == 1:
                        psA_g, psB_g, psC_g, psD_g = psA2, psB2, psC2, psD2
                        e3cols = 512
                    else:
                        psA_g, psB_g, psC_g, psD_g = psA, psB, psC, psD
                        e3cols = 512 if (streams == 2 or deep) else 1024
                    e1p = psB_g.tile([128, 512], f32, name="e1p", tag="e1p")
                    h2rs = []
                    h2p2 = h2r2 = None
                    for r in range(RG):
                        row = RG * g + r
                        # h1 = relu(U + V[:, row]) on GPSIMD (frees DVE/ACT)
                        h1 = h1pool.tile([128, M], w2dt, name="h1", tag="h1")
                        h1_eng = nc.gpsimd if h1_engine == "pool" else nc.vector
                        if ablate in ("h1", "allx"):
                            h1_eng.tensor_scalar(
                                h1[:, 0:4], Usb[:, 0:4], Vsb[:, row:row + 1],
                                0.0, ALU.add, ALU.max)
                        else:
                            h1_eng.tensor_scalar(
                                h1, Usb, Vsb[:, row:row + 1], 0.0,
                                ALU.add, ALU.max)
                        # eq layer 2 (block-diag over d)
                        if h2pair:
                            if r % 2 == 0:
                                h2p2 = psA_g.tile([128, 1024], f32,
                                                  name="h2p", tag="h2p")
                                h2r2 = h2rpool.tile([128, 1024], w2dt,
                                                    name="h2r", tag="h2r")
                            h2p = h2p2[:, 512 * (r % 2):512 * (r % 2 + 1)]
                        else:
                            h2p = psA_g.tile([128, 512], f32, name="h2p",
                                             tag="h2p")
                        if ablate in ("pe", "allx"):
                            nc.tensor.matmul(h2p[:, 0:4], W2sb, h1[:, 0:4],
                                             start=True, stop=True)
                        elif ablate == "ldw":
                            nc.tensor.matmul(h2p[0:4, :], W2sb[:, 0:4], h1,
                                             start=True, stop=True)
                        else:
                            nc.tensor.matmul(h2p, W2sb, h1, start=True, stop=True)
                        if h2pair:
                            h2r = h2r2[:, 512 * (r % 2):512 * (r % 2 + 1)]
                            if r % 2 == 1:
                                nc.scalar.activation(h2r2, h2p2, AF.Relu,
                                                     bias=B2sb)
                        else:
                            h2r = h2rpool.tile([128, 512], f32r, name="h2r",
                                               tag="h2r")
                            if ablate in ("act", "allx"):
                                nc.scalar.activation(h2r[:, 0:4], h2p[:, 0:4],
                                                     AF.Relu, bias=B2sb)
                            else:
                                nc.scalar.activation(h2r, h2p, AF.Relu,
                                                     bias=B2sb)
                        h2rs.append(h2r)
                    # eq layer 3 + sum_d + enc layer 1: pack the 4 group rows
                    # into partitions (r,k2) by accumulating block-sparse
                    # weights
                    for r in range(RG):
                        if ablate in ("pe", "allx"):
                            nc.tensor.matmul(
                                e1p[:, 0:4], WC4sb[:, 128 * r:128 * (r + 1)],
                                h2rs[r][:, 0:4],
                                start=(r == 0), stop=(r == RG - 1))
                        elif ablate == "ldw":
                            nc.tensor.matmul(
                                e1p[0:4, :], WC4sb[:, 128 * r:128 * r + 4],
                                h2rs[r],
                                start=(r == 0), stop=(r == RG - 1))
                        else:
                            nc.tensor.matmul(
                                e1p, WC4sb[:, 128 * r:128 * (r + 1)], h2rs[r],
                                start=(r == 0), stop=(r == RG - 1))
                    e1r = e1rpool.tile([128, 512], f32r, name="e1r", tag="e1r")
                    if ablate in ("act", "allx"):
                        nc.scalar.activation(e1r[:, 0:4], e1p[:, 0:4],
                                             AF.Relu, bias=B1sb)
                    else:
                        nc.scalar.activation(e1r, e1p, AF.Relu, bias=B1sb)
                    # enc layer 2, block-diag over the 4 packed rows
                    e2p = psC_g.tile([128, 512], f32, name="e2p", tag="e2p")
                    if ablate in ("pe", "allx"):
                        nc.tensor.matmul(e2p[:, 0:4], W24sb, e1r[:, 0:4],
                                         start=True, stop=True)
                    elif ablate == "ldw":
                        nc.tensor.matmul(e2p[0:4, :], W24sb[:, 0:4], e1r,
                                         start=True, stop=True)
                    else:
                        nc.tensor.matmul(e2p, W24sb, e1r, start=True, stop=True)
                    # e2relu always on DVE: keeps enc3's slot-release and
                    # producer waits on the single DVE semaphore
                    e2r = e2rpool.tile([128, 512], f32r, name="e2r", tag="e2r")
                    if ablate in ("act", "allx"):
                        nc.scalar.activation(e2r[:, 0:4], e2p[:, 0:4],
                                             AF.Relu, bias=B22sb)
                    elif e2_engine == "act" or (e2_engine == "alt" and g % 2 == 0):
                        nc.scalar.activation(e2r, e2p, AF.Relu, bias=B22sb)
                    else:
                        nc.vector.tensor_scalar(e2r, e2p, B22sb, 0.0,
                                                ALU.add, ALU.max)
                    # enc layer 3 back to [L=128, m=512]; two rows share a
                    # 2-bank PSUM tile -> one max-reduce per row pair
                    nhalf = RG // 2 if e3cols == 1024 else RG
                    rows_per_red = 2 if e3cols == 1024 else 1
                    for half in range(nhalf):
                        row = RG * g + rows_per_red * half
                        e3p = psD_g.tile([128, e3cols], f32, name="e3p",
                                         tag="e3p")
                        for i in range(rows_per_red):
                            r = rows_per_red * half + i
                            if ablate in ("pe", "allx"):
                                nc.tensor.matmul(
                                    e3p[:, 512 * i:512 * i + 4],
                                    E3sb[32 * r:32 * (r + 1), :],
                                    e2r[32 * r:32 * (r + 1), 0:4],
                                    start=True, stop=True,
                                    tile_position=(32 * r, 0))
                            elif ablate == "ldw":
                                nc.tensor.matmul(
                                    e3p[0:4, 512 * i:512 * (i + 1)],
                                    E3sb[32 * r:32 * (r + 1), 0:4],
                                    e2r[32 * r:32 * (r + 1), :],
                                    start=True, stop=True,
                                    tile_position=(32 * r, 0))
                            else:
                                nc.tensor.matmul(
                                    e3p[:, 512 * i:512 * (i + 1)],
                                    E3sb[32 * r:32 * (r + 1), :],
                                    e2r[32 * r:32 * (r + 1), :],
                                    start=True, stop=True,
                                    tile_position=(32 * r, 0))
                        red_in = (e3p[:, 0:8 * rows_per_red]
                                  if ablate in ("red", "allx") else e3p)
                        nc.vector.reduce_max(
                            out=outacc[:, row:row + rows_per_red],
                            in_=red_in.rearrange("p (r m) -> p r m",
                                                 r=rows_per_red),
                            axis=AX.X,
                        )

                # epilogue: add enc_b3, transpose [L,row] -> [row,L], store
                outb = outpool.tile([128, ROWS], f32, name="outb")
                nc.vector.tensor_scalar(outb, outacc, B3sb, None, ALU.add)
                for t in range(ROWS // 128):
                    tp = psC.tile([128, 128], f32, name="tp", tag="e2p")
                    nc.tensor.transpose(tp, outb[:, 128 * t:128 * (t + 1)], IDsb)
                    ot = h1pool.tile([128, 128], f32, name="ot", tag="h1")
                    nc.scalar.copy(ot, tp)
                    nc.sync.dma_start(out=OUT[128 * t:128 * (t + 1), :], in_=ot)

    nc.compile()
    return nc


def _get_program():
    global _PROG
    if _PROG is None:
        _PROG = _build_program(h1_engine="dve", e2_engine=E2_ENGINE, bf16_h1=BF16_H1)
    return _PROG


def _wc4(wc):
    """4 block-sparse copies of the [128,32] combo weight: block r lands in
    output partitions 32r..32r+32 when accumulated."""
    out = np.zeros((128, 4, 128), np.float32)
    for r in range(4):
        out[:, r, 32 * r:32 * (r + 1)] = wc
    return np.ascontiguousarray(out.reshape(128, 512))


def _derived_inputs(inputs):
    """Host-side prep: per-core U/V tiles + folded weight matrices."""
    f = lambda k: np.asarray(inputs[k], dtype=np.float32)
    X, Y = f("X"), f("Y")
    eq_w1, eq_b1 = f("eq_w1"), f("eq_b1")
    eq_w2, eq_b2 = f("eq_w2"), f("eq_b2")
    eq_w3, eq_b3 = f("eq_w3"), f("eq_b3")
    enc_w1, enc_b1 = f("enc_w1"), f("enc_b1")
    enc_w2, enc_b2 = f("enc_w2"), f("enc_b2")
    enc_w3, enc_b3 = f("enc_w3"), f("enc_b3")

    w1a = eq_w1[:, 0]  # multiplies Y
    w1c = eq_w1[:, 1]  # multiplies X

    # U[b] [(d,k), m] = w1a[k]*Y[b,m,d] + eq_b1[k]
    Yt = Y.transpose(0, 2, 1)  # (B, D, M)
    Uall = (w1a[None, None, :, None] * Yt[:, :, None, :]
            + eq_b1[None, None, :, None]).reshape(B, D * H, M)
    # V [(d,k), (b,n)] = w1c[k]*X[b,n,d]
    Xt = X.transpose(0, 2, 1)  # (B, D, N)
    Vall = (w1c[None, None, :, None] * Xt[:, :, None, :]).reshape(B, D * H, N)
    Vflat = np.concatenate([Vall[b] for b in range(B)], axis=1)  # (128, B*N)

    shared = {
        "W2BLK": np.ascontiguousarray(np.kron(np.eye(D, dtype=np.float32), eq_w2.T)),
        "WCOMBO4": _wc4(np.tile(eq_w3[0], D)[:, None] * enc_w1[:, 0][None, :]),
        "W2BLK4": np.ascontiguousarray(np.kron(np.eye(4, dtype=np.float32), enc_w2.T)),
        "ENC3T4": np.ascontiguousarray(np.tile(enc_w3.T, (4, 1))),
        "BCOL": np.ascontiguousarray(np.stack([
            np.tile(eq_b2, D),
            np.tile(enc_w1[:, 0] * (D * eq_b3[0]) + enc_b1, 4),
            np.tile(enc_b2, 4),
            enc_b3,
        ], axis=1)),
        "IDN": np.eye(128, dtype=np.float32),
    }
    shared = {k: v.astype(np.float32) for k, v in shared.items()}
    if BF16_H1:
        import ml_dtypes
        shared["W2BLK"] = shared["W2BLK"].astype(ml_dtypes.bfloat16)

    in_maps = []
    for c in range(NCORES):
        b = (c * ROWS) // N
        u = np.ascontiguousarray(Uall[b])
        if BF16_H1:
            import ml_dtypes
            u = u.astype(ml_dtypes.bfloat16)
        in_maps.append({
            "U": u,
            "V": np.ascontiguousarray(Vflat[:, c * ROWS:(c + 1) * ROWS]),
            **shared,
        })
    return in_maps


TRACE = False
LAST_RESULT = None


def kernel(**inputs) -> np.ndarray:
    global LAST_RESULT
    from concourse.bass_utils import run_bass_kernel_spmd

    nc = _get_program()
    in_maps = _derived_inputs(inputs)
    res = run_bass_kernel_spmd(
        nc, in_maps, list(range(NCORES)), trace=TRACE
    )
    LAST_RESULT = res
    out = np.concatenate([res.results[c]["OUT"] for c in range(NCORES)], axis=0)
    return out.reshape(B, N, L).astype(np.float32)


# revision 43
# speedup vs baseline: 1.0270x; 1.0270x over previous
# Trainium2 Bass kernel for nn_EquiRNBlock1 (gnn_message_passing).
#
# Reference computation (B=4, N=512, M=512, D=4, H=32, L=128):
#   pairs[b,n,m,d,:] = (Y[b,m,d], X[b,n,d])
#   h1 = relu(W1 @ pairs + b1)            # 2 -> 32, elementwise over (b,n,m,d)
#   h2 = relu(W2 @ h1 + b2)               # 32 -> 32
#   z  = w3 . h2 + b3                     # 32 -> 1
#   zs[b,n,m] = sum_d z                   # sum over D
#   e1 = relu(encw1 * zs + encb1)         # 1 -> 32
#   e2 = relu(encw2 @ e1 + encb2)         # 32 -> 32
#   e3 = encw3 @ e2 + encb3               # 32 -> 128
#   out[b,n,l] = max_m e3[b,n,m,l]
#
# Device mapping (8 cores, SPMD):
#   Flatten (b,n) -> 2048 rows; core c owns rows [256c, 256c+256) (each core
#   touches exactly one batch b = c//2).  Per (b,n) row the whole M=512 grid
#   is processed on-chip:
#     - Layer 1 splits algebraically: h1 = relu(U[b] + V[b,n]) with
#       U[(d,k),m] = W1[k,0]*Y[b,m,d] + b1[k] shared across rows and
#       V[(d,k)] = W1[k,1]*X[b,n,d] a per-partition scalar -> one GPSIMD
#       tensor_scalar (add, then max 0) per row.
#     - eq layer 2 = one 128x128 block-diag (over d) matmul per row.
#     - eq layer 3 + sum_d + enc layer 1 fold into one rank-1-combined
#       matmul; 4 rows pack into the 128 PSUM partitions via accumulation
#       of block-sparse weights (PE cannot write PSUM at a partition
#       offset, but it can accumulate full-height).
#     - enc layer 2 = one block-diag(4 rows) matmul per group.
#     - enc layer 3 reads the packed rows back out (rhs partition offsets
#       are legal) producing [L=128, m=512] per row; max over m = DVE
#       free-dim reduce, two rows per [128,1024] PSUM pair.
#   All matmul operands are float32r (TF32-like, 1 col/cycle at N>=256;
#   true fp32 streams at 1/4 rate).  Weights/activations round to f32r on
#   write, PSUM accumulates fp32.  Measured |rel err| ~6e-4 on hardware.
#   Outputs accumulate as [L, row] columns, PE-transposed at the end.

import numpy as np

B, N, M, D = 4, 512, 512, 4
H, L = 32, 128
NCORES = 8
ROWS = (B * N) // NCORES  # 256 rows per core
RG = 4                    # rows per pipeline group

_PROG = None
BF16_H1 = False
E2_ENGINE = "act"


def _build_program(loop_iters=None, h1_engine="dve", e2_engine="act", bf16_h1=False, ablate=None, streams=1, deep=False, h2pair=False):
    import contextlib
    import concourse.bacc as bacc
    import concourse.tile as tile
    import concourse.mybir as mybir

    f32 = mybir.dt.float32
    f32r = mybir.dt.float32r
    AF = mybir.ActivationFunctionType
    ALU = mybir.AluOpType
    AX = mybir.AxisListType

    import os
    nc = bacc.Bacc("TRN2", target_bir_lowering=False, debug=False, use_seq_codegen=os.environ.get("SEQCG", "") == "1")

    bf16 = mybir.dt.bfloat16
    uvdt = bf16 if bf16_h1 else f32
    w2dt = bf16 if bf16_h1 else f32r
    U = nc.dram_tensor("U", [128, M], uvdt, kind="ExternalInput").ap()
    V = nc.dram_tensor("V", [128, ROWS], f32, kind="ExternalInput").ap()
    W2BLK = nc.dram_tensor("W2BLK", [128, 128], w2dt, kind="ExternalInput").ap()
    WCOMBO4 = nc.dram_tensor("WCOMBO4", [128, 512], f32r, kind="ExternalInput").ap()
    W2BLK4 = nc.dram_tensor("W2BLK4", [128, 128], f32r, kind="ExternalInput").ap()
    ENC3T4 = nc.dram_tensor("ENC3T4", [128, 128], f32r, kind="ExternalInput").ap()
    BCOL = nc.dram_tensor("BCOL", [128, 4], f32, kind="ExternalInput").ap()
    IDN = nc.dram_tensor("IDN", [128, 128], f32, kind="ExternalInput").ap()
    OUT = nc.dram_tensor("OUT", [ROWS, 128], f32, kind="ExternalOutput").ap()

    with tile.TileContext(nc) as tc:
        with (
            tc.tile_pool(name="consts", bufs=1) as consts,
            tc.tile_pool(name="h1p", bufs=(4 if os.environ.get("SBUFS") == "1" else 6)) as h1pool,
            tc.tile_pool(name="h2rp", bufs=(5 if os.environ.get("SBUFS") == "1" else 8)) as h2rpool,
            tc.tile_pool(name="e1rp", bufs=3) as e1rpool,
            tc.tile_pool(name="e2rp", bufs=3) as e2rpool,
            tc.tile_pool(name="outp", bufs=1) as outpool,
            tc.tile_pool(name="psA", bufs=(1 if h2pair else (2 if streams == 1 else 1)),
                         space="PSUM") as psA,
            tc.tile_pool(name="psB", bufs=(2 if deep else 1), space="PSUM") as psB,
            tc.tile_pool(name="psC", bufs=(2 if deep else 1), space="PSUM") as psC,
            tc.tile_pool(name="psD", bufs=(2 if streams == 1 else 1),
                         space="PSUM") as psD,
            tc.tile_pool(name="psA2", bufs=1, space="PSUM") as psA2,
            tc.tile_pool(name="psB2", bufs=1, space="PSUM") as psB2,
            tc.tile_pool(name="psC2", bufs=1, space="PSUM") as psC2,
            tc.tile_pool(name="psD2", bufs=1, space="PSUM") as psD2,
        ):
            Usb = consts.tile_from(U, name="Usb")
            Vsb = consts.tile_from(V, name="Vsb")
            W2sb = consts.tile_from(W2BLK, name="W2sb")
            WC4sb = consts.tile_from(WCOMBO4, name="WC4sb")
            W24sb = consts.tile_from(W2BLK4, name="W24sb")
            E3sb = consts.tile_from(ENC3T4, name="E3sb")
            Bsb = consts.tile_from(BCOL, name="Bsb")
            IDsb = consts.tile_from(IDN, name="IDsb")
            B2sb = Bsb[:, 0:1]     # eq_b2 tiled over d
            B1sb = Bsb[:, 1:2]     # enc_w1*D*eq_b3 + enc_b1, tiled over rows
            B22sb = Bsb[:, 2:3]    # enc_b2 tiled over rows
            B3sb = Bsb[:, 3:4]     # enc_b3

            outacc = outpool.tile([128, ROWS], f32, name="outacc")

            loop_cm = (
                tc.For_i(0, loop_iters, 1,
                         hint_engines=(mybir.EngineType.PE,
                                       mybir.EngineType.Activation,
                                       mybir.EngineType.DVE,
                                       mybir.EngineType.Pool))
                if loop_iters is not None else contextlib.nullcontext()
            )
            with loop_cm:
                if streams == 2:
                    group_order = [2 * gp + s for gp in range(ROWS // RG // 2)
                                   for s in range(2)]
                else:
                    group_order = list(range(ROWS // RG))
                for g in group_order:
                    if streams == 2 and g % 2 == 1:
                        psA_g, psB_g, psC_g, psD_g = psA2, psB2, psC2, psD2
                        e3cols = 512
                    else:
                        psA_g, psB_g, psC_g, psD_g = psA, psB, psC, psD
                        e3cols = 512 if (streams == 2 or deep) else 1024
                    e1p = psB_g.tile([128, 512], f32, name="e1p", tag="e1p")
                    h2rs = []
                    h2p2 = h2r2 = None
                    for r in range(RG):
                        row = RG * g + r
                        # h1 = relu(U + V[:, row]) on GPSIMD (frees DVE/ACT)
                        h1 = h1pool.tile([128, M], w2dt, name="h1", tag="h1")
                        h1_eng = nc.gpsimd if h1_engine == "pool" else nc.vector
                        if ablate in ("h1", "allx"):
                            h1_eng.tensor_scalar(
                                h1[:, 0:4], Usb[:, 0:4], Vsb[:, row:row + 1],
                                0.0, ALU.add, ALU.max)
                        else:
                            h1_eng.tensor_scalar(
                                h1, Usb, Vsb[:, row:row + 1], 0.0,
                                ALU.add, ALU.max)
                        # eq layer 2 (block-diag over d)
                        if h2pair:
                            if r % 2 == 0:
                                h2p2 = psA_g.tile([128, 1024], f32,
                                                  name="h2p", tag="h2p")
                                h2r2 = h2rpool.tile([128, 1024], w2dt,
                                                    name="h2r", tag="h2r")
                            h2p = h2p2[:, 512 * (r % 2):512 * (r % 2 + 1)]
                        else:
                            h2p = psA_g.tile([128, 512], f32, name="h2p",
                                             tag="h2p")
                        if ablate in ("pe", "allx"):
                            nc.tensor.matmul(h2p[:, 0:4], W2sb, h1[:, 0:4],
                                             start=True, stop=True)
                        elif ablate == "ldw":
                            nc.tensor.matmul(h2p[0:4, :], W2sb[:, 0:4], h1,
                                             start=True, stop=True)
                        else:
                            nc.tensor.matmul(h2p, W2sb, h1, start=True, stop=True)
                        if h2pair:
                            h2r = h2r2[:, 512 * (r % 2):512 * (r % 2 + 1)]
                            if r % 2 == 1:
                                nc.scalar.activation(h2r2, h2p2, AF.Relu,
                                                     bias=B2sb)
                        else:
                            h2r = h2rpool.tile([128, 512], f32r, name="h2r",
                                               tag="h2r")
                            if ablate in ("act", "allx"):
                                nc.scalar.activation(h2r[:, 0:4], h2p[:, 0:4],
                                                     AF.Relu, bias=B2sb)
                            else:
                                nc.scalar.activation(h2r, h2p, AF.Relu,
                                                     bias=B2sb)
                        h2rs.append(h2r)
                    # eq layer 3 + sum_d + enc layer 1: pack the 4 group rows
                    # into partitions (r,k2) by accumulating block-sparse
                    # weights
                    for r in range(RG):
                        if ablate in ("pe", "allx"):
                            nc.tensor.matmul(
                                e1p[:, 0:4], WC4sb[:, 128 * r:128 * (r + 1)],
                                h2rs[r][:, 0:4],
                                start=(r == 0), stop=(r == RG - 1))
                        elif ablate == "ldw":
                            nc.tensor.matmul(
                                e1p[0:4, :], WC4sb[:, 128 * r:128 * r + 4],
                                h2rs[r],
                                start=(r == 0), stop=(r == RG - 1))
                        else:
                            nc.tensor.matmul(
                                e1p, WC4sb[:, 128 * r:128 * (r + 1)], h2rs[r],
                                start=(r == 0), stop=(r == RG - 1))
                    e1r = e1rpool.tile([128, 512], f32r, name="e1r", tag="e1r")
                    if ablate in ("act", "allx"):
                        nc.scalar.activation(e1r[:, 0:4], e1p[:, 0:4],
                                             AF.Relu, bias=B1sb)
                    else:
                        nc.scalar.activation(e1r, e1p, AF.Relu, bias=B1sb)
                    # enc layer 2, block-diag over the 4 packed rows
                    e2p = psC_g.tile([128, 512], f32, name="e2p", tag="e2p")
                    if ablate in ("pe", "allx"):
                        nc.tensor.matmul(e2p[:, 0:4], W24sb, e1r[:, 0:4],
                                         start=True, stop=True)
                    elif ablate == "ldw":
                        nc.tensor.matmul(e2p[0:4, :], W24sb[:, 0:4], e1r,
                                         start=True, stop=True)
                    else:
                        nc.tensor.matmul(e2p, W24sb, e1r, start=True, stop=True)
                    # e2relu always on DVE: keeps enc3's slot-release and
                    # producer waits on the single DVE semaphore
                    e2r = e2rpool.tile([128, 512], f32r, name="e2r", tag="e2r")
                    if ablate in ("act", "allx"):
                        nc.scalar.activation(e2r[:, 0:4], e2p[:, 0:4],
                                             AF.Relu, bias=B22sb)
                    elif e2_engine == "act" or (e2_engine == "alt" and g % 2 == 0):
                        nc.scalar.activation(e2r, e2p, AF.Relu, bias=B22sb)
                    else:
                        nc.vector.tensor_scalar(e2r, e2p, B22sb, 0.0,
                                                ALU.add, ALU.max)
                    # enc layer 3 back to [L=128, m=512]; two rows share a
                    # 2-bank PSUM tile -> one max-reduce per row pair
                    nhalf = RG // 2 if e3cols == 1024 else RG
                    rows_per_red = 2 if e3cols == 1024 else 1
                    for half in range(nhalf):
                        row = RG * g + rows_per_red * half
                        e3p = psD_g.tile([128, e3cols], f32, name="e3p",
                                         tag="e3p")
                        for i in range(rows_per_red):
                            r = rows_per_red * half + i
                            if ablate in ("pe", "allx"):
                                nc.tensor.matmul(
                                    e3p[:, 512 * i:512 * i + 4],
                                    E3sb[32 * r:32 * (r + 1), :],
                                    e2r[32 * r:32 * (r + 1), 0:4],
                                    start=True, stop=True,
                                    tile_position=(32 * r, 0))
                            elif ablate == "ldw":
                                nc.tensor.matmul(
                                    e3p[0:4, 512 * i:512 * (i + 1)],
                                    E3sb[32 * r:32 * (r + 1), 0:4],
                                    e2r[32 * r:32 * (r + 1), :],
                                    start=True, stop=True,
                                    tile_position=(32 * r, 0))
                            else:
                                nc.tensor.matmul(
                                    e3p[:, 512 * i:512 * (i + 1)],
                                    E3sb[32 * r:32 * (r + 1), :],
                                    e2r[32 * r:32 * (r + 1), :],
                                    start=True, stop=True,
                                    tile_position=(32 * r, 0))
                        red_in = (e3p[:, 0:8 * rows_per_red]
                                  if ablate in ("red", "allx") else e3p)
                        nc.vector.reduce_max(
                            out=outacc[:, row:row + rows_per_red],
                            in_=red_in.rearrange("p (r m) -> p r m",
                                                 r=rows_per_red),
                            axis=AX.X,
                        )

                # epilogue: add enc_b3, transpose [L,row] -> [row,L], store
                outb = outpool.tile([128, ROWS], f32, name="outb")
                nc.vector.tensor_scalar(outb, outacc, B3sb, None, ALU.add)
                for t in range(ROWS // 128):
                    tp = psC.tile([128, 128], f32, name="tp", tag="e2p")
                    nc.tensor.transpose(tp, outb[:, 128 * t:128 * (t + 1)], IDsb)
                    ot = h1pool.tile([128, 128], f32, name="ot", tag="h1")
                    nc.scalar.copy(ot, tp)
                    nc.sync.dma_start(out=OUT[128 * t:128 * (t + 1), :], in_=ot)

    nc.compile()
    return nc


def _get_program():
    global _PROG
    if _PROG is None:
        _PROG = _build_program(h1_engine="dve", e2_engine=E2_ENGINE, bf16_h1=BF16_H1)
    return _PROG


def _wc4(wc):
    """4 block-sparse copies of the [128,32] combo weight: block r lands in
    output partitions 32r..32r+32 when accumulated."""
    out = np.zeros((128, 4, 128), np.float32)
    for r in range(4):
        out[:, r, 32 * r:32 * (r + 1)] = wc
    return np.ascontiguousarray(out.reshape(128, 512))


def _derived_inputs(inputs):
    """Host-side prep: per-core U/V tiles + folded weight matrices."""
    f = lambda k: np.asarray(inputs[k], dtype=np.float32)
    X, Y = f("X"), f("Y")
    eq_w1, eq_b1 = f("eq_w1"), f("eq_b1")
    eq_w2, eq_b2 = f("eq_w2"), f("eq_b2")
    eq_w3, eq_b3 = f("eq_w3"), f("eq_b3")
    enc_w1, enc_b1 = f("enc_w1"), f("enc_b1")
    enc_w2, enc_b2 = f("enc_w2"), f("enc_b2")
    enc_w3, enc_b3 = f("enc_w3"), f("enc_b3")

    w1a = eq_w1[:, 0]  # multiplies Y
    w1c = eq_w1[:, 1]  # multiplies X

    # U[b] [(d,k), m] = w1a[k]*Y[b,m,d] + eq_b1[k]
    Yt = Y.transpose(0, 2, 1)  # (B, D, M)
    Uall = (w1a[None, None, :, None] * Yt[:, :, None, :]
            + eq_b1[None, None, :, None]).reshape(B, D * H, M)
    # V [(d,k), (b,n)] = w1c[k]*X[b,n,d]
    Xt = X.transpose(0, 2, 1)  # (B, D, N)
    Vall = (w1c[None, None, :, None] * Xt[:, :, None, :]).reshape(B, D * H, N)
    Vflat = np.concatenate([Vall[b] for b in range(B)], axis=1)  # (128, B*N)

    shared = {
        "W2BLK": np.ascontiguousarray(np.kron(np.eye(D, dtype=np.float32), eq_w2.T)),
        "WCOMBO4": _wc4(np.tile(eq_w3[0], D)[:, None] * enc_w1[:, 0][None, :]),
        "W2BLK4": np.ascontiguousarray(np.kron(np.eye(4, dtype=np.float32), enc_w2.T)),
        "ENC3T4": np.ascontiguousarray(np.tile(enc_w3.T, (4, 1))),
        "BCOL": np.ascontiguousarray(np.stack([
            np.tile(eq_b2, D),
            np.tile(enc_w1[:, 0] * (D * eq_b3[0]) + enc_b1, 4),
            np.tile(enc_b2, 4),
            enc_b3,
        ], axis=1)),
        "IDN": np.eye(128, dtype=np.float32),
    }
    shared = {k: v.astype(np.float32) for k, v in shared.items()}
    if BF16_H1:
        import ml_dtypes
        shared["W2BLK"] = shared["W2BLK"].astype(ml_dtypes.bfloat16)

    in_maps = []
    for c in range(NCORES):
        b = (c * ROWS) // N
        u = np.ascontiguousarray(Uall[b])
        if BF16_H1:
            import ml_dtypes
            u = u.astype(ml_dtypes.bfloat16)
        in_maps.append({
            "U": u,
            "V": np.ascontiguousarray(Vflat[:, c * ROWS:(c + 1) * ROWS]),
            **shared,
        })
    return in_maps


TRACE = False
LAST_RESULT = None


def kernel(**inputs) -> np.ndarray:
    global LAST_RESULT
    from concourse.bass_utils import run_bass_kernel_spmd

    nc = _get_program()
    in_maps = _derived_inputs(inputs)
    res = run_bass_kernel_spmd(
        nc, in_maps, list(range(NCORES)), trace=TRACE
    )
    LAST_RESULT = res
    out = np.concatenate([res.results[c]["OUT"] for c in range(NCORES)], axis=0)
    return out.reshape(B, N, L).astype(np.float32)
